# revision 22
# baseline (speedup 1.0000x reference)
"""Multi-head causal attention (B=8, S=1024, E=512, H=8, Dk=Dv=64) on 8 NeuronCores.

Sharding: data-parallel over batch. Core b computes the full attention block
for X[b]; no collectives. Host pre-transposes X[b] -> [E, S], converts matmul
operands to bf16, and packs weights into wide slabs so the device issues only
a few input DMAs (sync + scalar HW queues in compute order).

Per-core dataflow (bf16 matmuls, fp32 PSUM accumulate / softmax math):
  XT slab [128, qc*2048 + c*512 + s] resident in SBUF
  QT/KT per head-pair per q-half: [128 dd, 512 q] (W chunk stationary);
      all QTs emitted before all KTs so the PE never queues behind the WK
      DMA; Q/K bias applied on ScalarE (Identity + per-partition bias AP)
  V = (X @ Wv + bv) -> 8 tiles [128 s, 8*(64+1)] with a ones column per head
      so the AV matmul also emits softmax denominators
  attention per q-chunk as ONE flat block stream across all 4 head-pairs,
  software-pipelined one k-block ahead (the pipeline crosses pair
  boundaries, so the PE never drains at a pair switch):
    score^T blocks [128 k, 2x512 q] on PE (k-blocks above the diagonal
    skipped, partial blocks column-trimmed), causal triangle fixed up on the
    PE itself (-1e9*I @ tri01), exp on ScalarE (scale=1/8 folded),
    AV accum O^T[65, q] with denom row 64.
  finalize per pair: copy denom row PSUM->SBUF (ScalarE for the last
    processed pair of each phase, DVE otherwise), reciprocal_approx_fast on
    DVE, GpSimd partition-broadcast, DVE scale; head-pair upper half placed
    via SBUF->SBUF DMA -- except the LAST PROCESSED pair of phase 1
    (head-pair 2; phase-1 pairs run in order 3,0,1,2), whose projection
    reads the scaled halves directly via two K=64 matmuls, so no DMA sits
    on the tail-critical path.
  Y[s-chunk] = sum_p O_pair^T-block^T @ Wo: si 0-3 emitted during the
  interphase (overlapping qc=1 attention), si 4-7 at the tail with
  pair-0/1/3 contributions emitted ahead of the finalize-gated pair-2
  splits; yp4/5 reuse the score-pipeline PSUM slots. The y copy applies
  +bo via scalar_tensor_tensor on DVE, and bf16 y stores stream out on
  both the sync and scalar HW queues; the host converts back to fp32.
"""

import numpy as np
import ml_dtypes

import concourse.bass as bass
import concourse.tile as tile
import concourse.mybir as mybir
from concourse import bacc
from concourse import bass_utils

B, S, E = 8, 1024, 512
H, DK, DV = 8, 64, 64
HD = H * DK  # 512
P = 128
EC = E // P  # 4 contraction chunks over E
NPAIR = H // 2
NCORES = 8
F32 = mybir.dt.float32
BF16 = mybir.dt.bfloat16
NEG = -1.0e9

_COMPILED = None

# phase-1 pair processing order: pair 2 last (it feeds the tail-path splits)
P1_ORDER = (3, 0, 1, 2)
TAIL_PAIR = P1_ORDER[-1]


def _body(nc, tc, const, work, ps, pb, d):
    # ---- const tiles + packed input DMAs ----
    XT = const.tile([P, 4096], BF16, tag="xt", name="XT")
    WQ = const.tile([P, 2048], BF16, tag="wq", name="WQ")   # (pair, c, dd)
    WK = const.tile([P, 2048], BF16, tag="wk", name="WK")   # (pair, c, dd)
    WV = const.tile([P, 2048], BF16, tag="wv", name="WV")   # (c, hd)
    WO = const.tile([P, 2048], BF16, tag="wo", name="WO")   # (c, e)
    CB = const.tile([P, 2176], BF16, tag="cb", name="CB")
    FB = const.tile([P, 8], F32, tag="fb", name="FB")

    # sync queue carries the compute-gating chain; scalar queue the rest
    nc.sync.dma_start(WQ[:, 0:1024], d["wq"][:, 0:1024])
    nc.sync.dma_start(XT[:, 0:1024], d["xt"][:, 0:1024])
    nc.sync.dma_start(XT[:, 1024:2048], d["xt"][:, 1024:2048])
    nc.sync.dma_start(WK[:, 0:1024], d["wk"][:, 0:1024])
    nc.sync.dma_start(WV[:], d["wv"][:])
    nc.sync.dma_start(XT[:, 2048:4096], d["xt"][:, 2048:4096])
    nc.sync.dma_start(WO[:], d["wo"][:])
    nc.scalar.dma_start(FB[:], d["fb"][:])
    nc.scalar.dma_start(WQ[:, 1024:2048], d["wq"][:, 1024:2048])
    nc.scalar.dma_start(WK[:, 1024:2048], d["wk"][:, 1024:2048])
    nc.scalar.dma_start(CB[:], d["cb"][:])

    bq_t = FB[:, 0:4]
    bk_t = FB[:, 4:8]
    negi = CB[:, 0:128]
    tri2 = CB[:, 128:384]
    bvb = CB[:, 384:896]
    bob = CB[:, 896:1408]
    # Wo rows for the tail pair's upper head, re-homed at partition base 0
    wo3b = CB[0:DV, 1408:1920]
    # keep-mask (k <= q), duplicated for both heads of a pair
    keep2 = CB[:, 1920:2176]

    qt, kt, ot_sb = {}, {}, {}
    ot_tmp = None
    v_sb = [None] * 8

    def emit_qk_one(p, qc, which, dve_copy=False):
        W, bias, store, nm = ((WQ, bq_t, qt, "q") if which == "q"
                              else (WK, bk_t, kt, "k"))
        pp = ps.tile([P, 512], F32, tag="ps", name=f"{nm}p{p}{qc}")
        for c in range(EC):
            nc.tensor.matmul(
                pp[:], W[:, c * 512 + p * P:c * 512 + (p + 1) * P],
                XT[:, qc * 2048 + c * 512:qc * 2048 + (c + 1) * 512],
                start=(c == 0), stop=(c == EC - 1))
        t = const.tile([P, 512], BF16, tag=f"{nm}t{p}{qc}", name=f"{nm}t{p}{qc}")
        if dve_copy:
            # injected mid-phase-1: ACT is the exp bottleneck there, so the
            # PSUM->SBUF bias-copy goes to DVE instead
            nc.vector.tensor_scalar_add(t[:], pp[:], bias[:, p:p + 1])
        else:
            nc.scalar.activation(
                t[:], pp[:], mybir.ActivationFunctionType.Identity,
                bias=bias[:, p:p + 1])
        store[p, qc] = t

    def emit_qkt(p, qc, dve_copy=False):
        emit_qk_one(p, qc, "q", dve_copy)
        emit_qk_one(p, qc, "k", dve_copy)

    def emit_v(si):
        qc, sl = divmod(si, 4)
        vp = ps.tile([P, HD], F32, tag="ps", name=f"vp{si}")
        for c in range(EC):
            base = qc * 2048 + c * 512
            nc.tensor.matmul(
                vp[:], XT[:, base + sl * P:base + (sl + 1) * P],
                WV[:, c * 512:(c + 1) * 512],
                start=(c == 0), stop=(c == EC - 1))
        t = const.tile([P, H * 65], BF16, tag=f"v{si}", name=f"v{si}")
        t3 = t.rearrange("p (h c) -> p h c", c=65)
        nc.gpsimd.memset(t[:], 1.0)  # contiguous; leaves the per-head ones column
        nc.vector.tensor_add(
            t3[:, :, 0:DV],
            vp.rearrange("p (h c) -> p h c", c=DV),
            bvb.rearrange("p (h c) -> p h c", c=DV))
        v_sb[si] = t

    def emit_y(si, yp):
        yo = work.tile([P, E], BF16, tag="yo", name=f"yo{si}", bufs=4)
        nc.vector.scalar_tensor_tensor(yo[:], yp[:], 1.0, bob[:],
                                       mybir.AluOpType.mult, mybir.AluOpType.add)
        q = nc.sync if si % 2 == 0 else nc.scalar
        q.dma_start(d["y"][si * P:(si + 1) * P, :], yo[:])

    def emit_proj(si):
        qc, sl = divmod(si, 4)
        yp = ps.tile([P, E], F32, tag="ps", name=f"yp{si}")
        for p in range(NPAIR):
            nc.tensor.matmul(
                yp[:], ot_sb[p, qc][:, sl * P:(sl + 1) * P],
                WO[:, p * 512:(p + 1) * 512],
                start=(p == 0), stop=(p == NPAIR - 1), skip_group_check=True)
        emit_y(si, yp)

    def emit_proj_tail():
        # si 4..7: pairs 0/1/3 emitted first (they overlap the last-processed
        # pair-2 finalize chain), then pair 2 via two K=64 matmuls straight
        # from the scaled halves (no SBUF->SBUF DMA on the tail path).
        yps = {}
        for si in range(4, 8):
            pool, tag = (pb, "st") if si < 6 else (ps, "ps")
            yps[si] = pool.tile([P, E], F32, tag=tag, name=f"yp{si}")

        def part(si, p, start):  # one non-tail pair contribution, K=128
            sl = si - 4
            nc.tensor.matmul(
                yps[si][:], ot_sb[p, 1][:, sl * P:(sl + 1) * P],
                WO[:, p * 512:(p + 1) * 512],
                start=start, stop=False, skip_group_check=True)

        def split(si, hb):  # tail-pair halves, K=64 from the scaled temps
            sl = si - 4
            tp = TAIL_PAIR
            if hb == 0:
                nc.tensor.matmul(
                    yps[si][:], ot_sb[tp, 1][0:DV, sl * P:(sl + 1) * P],
                    WO[0:DV, tp * 512:(tp + 1) * 512],
                    start=False, stop=False, tile_position=(0, 0),
                    skip_group_check=True)
            else:
                nc.tensor.matmul(
                    yps[si][:], ot_tmp[:, sl * P:(sl + 1) * P],
                    wo3b[:],
                    start=False, stop=True, tile_position=(0, 0),
                    skip_group_check=True)

        others = [p for p in range(NPAIR) if p != TAIL_PAIR]
        for si in (4, 5, 6, 7):
            for i, p in enumerate(others):
                part(si, p, i == 0)
        for si_pair in ((4, 5), (6, 7)):
            for si in si_pair:
                split(si, 0)
            for si in si_pair:
                split(si, 1)
        for si in range(4, 8):
            emit_y(si, yps[si])

    otps = {}

    def finalize(p, qc, act_copy=False):
        # ---- O^T *= 1/denom (reciprocal needs an SBUF input on HW, so the
        # PSUM denom row is copied first -- on ACT for the last pair of a
        # phase, where ACT is about to idle and the chain gates psum reuse)
        otp = otps[p, qc]
        tail_pair = (qc == 1 and p == TAIL_PAIR)
        ot = const.tile([P, 512], BF16, tag=f"ot{p}{qc}", name=f"ot{p}{qc}")
        rb = {}
        for hb in (0, 1):
            rrow = work.tile([1, 512], F32, tag="rrow", name=f"rrow{p}{qc}{hb}",
                             bufs=2)
            if act_copy:
                nc.scalar.activation(rrow[:], otp[hb][DV:DV + 1, :],
                                     mybir.ActivationFunctionType.Copy)
            else:
                nc.vector.tensor_copy(rrow[:], otp[hb][DV:DV + 1, :])
            rec = work.tile([1, 512], F32, tag="rec", name=f"rec{p}{qc}{hb}",
                            bufs=2)
            nc.vector.reciprocal_approx_fast(rec[:], rrow[:])
            rb[hb] = work.tile([DV, 512], F32, tag="rb", name=f"rb{p}{qc}{hb}",
                               bufs=2)
            nc.gpsimd.partition_broadcast(rb[hb][:], rec[:])
        nc.vector.tensor_mul(ot[0:DV, :], otp[0][0:DV, :], rb[0][:])
        # DVE cannot shift partitions: scale into a temp at base 0, then
        # SBUF->SBUF DMA into partitions 64-127 of the pair tile (skipped
        # for the tail pair, whose projection reads the temp directly)
        tmp = work.tile([DV, 512], BF16, tag="ottmp", name=f"ottmp{p}{qc}",
                        bufs=2)
        nc.vector.tensor_mul(tmp[:], otp[1][0:DV, :], rb[1][:])
        if tail_pair:
            nonlocal ot_tmp
            ot_tmp = tmp
        else:
            nc.sync.dma_start(ot[DV:P, :], tmp[:])
        ot_sb[p, qc] = ot

    def attn_phase(qc, inject=None):
        # flat block stream across all pairs with a TWO-block AV lookahead:
        # AV(i) issues after scores(i+1), scores(i+2) are queued, so the PE
        # has ~2 score blocks of runway over the exp latency
        n_ki = 4 * (qc + 1)
        order = P1_ORDER if qc == 1 else tuple(range(NPAIR))
        blocks = [(p, ki) for p in order for ki in range(n_ki)]
        stps, stes = {}, {}

        def emit_score(p, ki):
            kc, kl = divmod(ki, 4)
            diag = (ki * P - qc * 512) >= 0
            off = max(ki * P - qc * 512, 0)
            # qc=1 diag blocks are masked post-exp on DVE instead; qc=0 keeps
            # the PE fixup since the DVE would gate the exp pipeline
            pe_fix = diag and qc == 0
            stp = pb.tile([P, 1024], F32, tag="st", name=f"st{p}{qc}{ki}")
            for hb in (0, 1):
                hp = slice(hb * DK, (hb + 1) * DK)
                nc.tensor.matmul(
                    stp[:, hb * 512 + off:(hb + 1) * 512],
                    kt[p, kc][hp, kl * P:(kl + 1) * P],
                    qt[p, qc][hp, off:],
                    start=True, stop=not pe_fix, tile_position=(hb * DK, 0),
                    skip_group_check=True)
            if pe_fix:
                for hb in (0, 1):
                    nc.tensor.matmul(
                        stp[:, hb * 512 + off:hb * 512 + off + P],
                        negi[:], tri2[:, 0:P],
                        start=False, stop=True, skip_group_check=True)
            stps[p, ki] = (stp, off, diag and qc == 1)

        def emit_exp(p, ki):
            stp, off, dve_mask = stps[p, ki]
            ste = work.tile([P, 1024], BF16, tag="ste", name=f"ste{p}{qc}{ki}",
                            bufs=3)
            if off == 0:
                nc.scalar.activation(
                    ste[:], stp[:], mybir.ActivationFunctionType.Exp,
                    scale=0.125)
            else:
                stp3 = stp.rearrange("p (h q) -> p h q", h=2)[:, :, off:]
                ste3 = ste.rearrange("p (h q) -> p h q", h=2)[:, :, off:]
                nc.scalar.activation(
                    ste3, stp3, mybir.ActivationFunctionType.Exp, scale=0.125)
            if dve_mask:
                # zero exp'd scores above the diagonal (both heads at once)
                sv = ste.rearrange("p (h q) -> p h q", h=2)[:, :, off:off + P]
                nc.vector.tensor_mul(
                    sv, sv, keep2.rearrange("p (h q) -> p h q", h=2))
            stes[p, ki] = (ste, off)

        def emit_av(p, ki):
            ste, off = stes[p, ki]
            st_f, sp_f = (ki == 0), (ki == n_ki - 1)
            for hb in (0, 1):
                h = 2 * p + hb
                nc.tensor.matmul(
                    otps[p, qc][hb][:, off:], v_sb[ki][:, h * 65:h * 65 + 65],
                    ste[:, hb * 512 + off:(hb + 1) * 512],
                    start=st_f, stop=sp_f, skip_group_check=True)

        for idx, (p, ki) in enumerate(blocks):
            if ki == 0:
                otps[p, qc] = [ps.tile([DV + 1, 512], F32, tag="ps",
                                       name=f"otp{p}{qc}{hb}") for hb in (0, 1)]
            emit_score(p, ki)
            emit_exp(p, ki)
            if idx >= 1:
                pp, pk = blocks[idx - 1]
                emit_av(pp, pk)
                if pk == n_ki - 1:
                    finalize(pp, qc, act_copy=(pp == order[-1]))
                    if inject and pp in inject:
                        inject[pp]()
        emit_av(*blocks[-1])
        finalize(order[-1], qc, act_copy=True)

    # ---- emission schedule (ps-tag rotation keeps otp pairs ping-ponging
    # between bank pairs) ----
    for p in range(NPAIR):            # ps allocs 0-7; all QTs first so the
        emit_qk_one(p, 0, "q")        # PE never queues behind the WK DMA
    for p in range(NPAIR):
        emit_qk_one(p, 0, "k")
    for si in range(4):               # 8-11
        emit_v(si)
    attn_phase(0)                     # 12-19 -> slots (0,1)/(2,3)/(0,1)/(2,3)
    emit_qkt(0, 1)                    # 20-21
    emit_qkt(1, 1)                    # 22-23
    emit_v(4)                         # 24
    emit_v(5)                         # 25
    emit_qkt(2, 1)                    # 26-27
    emit_qkt(3, 1)                    # 28-29
    emit_v(6)                         # 30
    emit_v(7)                         # 31
    for si in range(4):               # (needs all qc=0 ot, finalized)
        emit_proj(si)
    attn_phase(1)                     # 36-43 -> slots (0,1)/(2,3)/(0,1)/(2,3)
    emit_proj_tail()                  # yp6/7 at ps 44,45 -> slots 0,1


def _build():
    nc = bacc.Bacc("TRN2", target_bir_lowering=False, debug=False)
    d = {
        "xt": nc.dram_tensor("xt", [P, 4096], BF16, kind="ExternalInput").ap(),
        "wq": nc.dram_tensor("wq", [P, 2048], BF16, kind="ExternalInput").ap(),
        "wk": nc.dram_tensor("wk", [P, 2048], BF16, kind="ExternalInput").ap(),
        "wv": nc.dram_tensor("wv", [P, 2048], BF16, kind="ExternalInput").ap(),
        "wo": nc.dram_tensor("wo", [P, 2048], BF16, kind="ExternalInput").ap(),
        "cb": nc.dram_tensor("cb", [P, 2176], BF16, kind="ExternalInput").ap(),
        "fb": nc.dram_tensor("fb", [P, 8], F32, kind="ExternalInput").ap(),
        "y": nc.dram_tensor("y", [S, E], BF16, kind="ExternalOutput").ap(),
    }
    with tile.TileContext(nc) as tc:
        with tc.tile_pool(name="const", bufs=1) as const, \
             tc.tile_pool(name="work", bufs=3) as work, \
             tc.tile_pool(name="ps", bufs=4, space="PSUM") as ps, \
             tc.tile_pool(name="pb", bufs=2, space="PSUM") as pb:
            _body(nc, tc, const, work, ps, pb, d)
    nc.compile()
    return nc


def get_nc():
    global _COMPILED
    if _COMPILED is None:
        _COMPILED = _build()
    return _COMPILED


def _prep_in_maps(X, Wq, bq, Wk, bk, Wv, bv, Wo, bo):
    f = np.float32
    bf = ml_dtypes.bfloat16

    def wslab(W):  # [H,E,Dk] -> [128, c*512 + (h*64+d)]
        Wr = np.transpose(np.asarray(W, f), (1, 0, 2)).reshape(E, HD)
        return np.ascontiguousarray(
            Wr.reshape(EC, P, HD).transpose(1, 0, 2).reshape(P, EC * HD).astype(bf))

    shared = {
        "wq": wslab(Wq),
        "wk": wslab(Wk),
        "wv": wslab(Wv),
        "wo": np.ascontiguousarray(
            np.asarray(Wo, f).reshape(EC, P, E).transpose(1, 0, 2)
            .reshape(P, EC * E).astype(bf)),
    }
    bq_t = np.asarray(bq, f).reshape(HD).reshape(NPAIR, P).T
    bk_t = np.asarray(bk, f).reshape(HD).reshape(NPAIR, P).T
    bvb = np.broadcast_to(np.asarray(bv, f).reshape(1, HD), (P, HD)).astype(bf)
    bob = np.broadcast_to(np.asarray(bo, f).reshape(1, E), (P, E)).astype(bf)
    kk = np.arange(P)[:, None]
    jj = np.arange(P)[None, :]
    shared["fb"] = np.ascontiguousarray(
        np.concatenate([bq_t, bk_t], axis=1).astype(f))
    negi = (np.eye(P, dtype=f) * NEG).astype(bf)
    tri01 = (kk > jj).astype(bf)
    # Wo rows for the tail pair's upper head, re-homed to partitions 0:64
    wo3b = np.zeros((P, E), dtype=bf)
    wo3b[0:DV] = np.asarray(Wo, f)[(2 * TAIL_PAIR + 1) * DV:
                                   (2 * TAIL_PAIR + 2) * DV, :].astype(bf)
    keep01 = (kk <= jj).astype(bf)
    shared["cb"] = np.ascontiguousarray(
        np.concatenate([negi, tri01, tri01, bvb, bob, wo3b, keep01, keep01],
                       axis=1))

    Xf = np.asarray(X, f)
    in_maps = []
    for b in range(B):
        m = dict(shared)
        # xt slab: [128, qc*2048 + c*512 + s']
        m["xt"] = np.ascontiguousarray(
            Xf[b].T.reshape(EC, P, 2, 512).transpose(1, 2, 0, 3)
            .reshape(P, 4096).astype(bf))
        in_maps.append(m)
    return in_maps


def kernel(X, Wq, bq, Wk, bk, Wv, bv, Wo, bo):
    nc = get_nc()
    in_maps = _prep_in_maps(X, Wq, bq, Wk, bk, Wv, bv, Wo, bo)
    res = bass_utils.run_bass_kernel_spmd(nc, in_maps, core_ids=list(range(NCORES)))
    return np.stack([res.results[b]["y"] for b in range(B)], axis=0).astype(np.float32)


def run_traced(X, Wq, bq, Wk, bk, Wv, bv, Wo, bo):
    """Like kernel() but with NTFF profiling; returns (out, exec_time_ns)."""
    nc = get_nc()
    in_maps = _prep_in_maps(X, Wq, bq, Wk, bk, Wv, bv, Wo, bo)
    res = bass_utils.run_bass_kernel_spmd(
        nc, in_maps, core_ids=list(range(NCORES)), trace=True)
    out = np.stack([res.results[b]["y"] for b in range(B)], axis=0).astype(np.float32)
    return out, res.exec_time_ns


# revision 23
# speedup vs baseline: 1.0077x; 1.0077x over previous
"""Multi-head causal attention (B=8, S=1024, E=512, H=8, Dk=Dv=64) on 8 NeuronCores.

Sharding: data-parallel over batch. Core b computes the full attention block
for X[b]; no collectives. Host pre-transposes X[b] -> [E, S], converts matmul
operands to bf16, and packs weights into wide slabs so the device issues only
a few input DMAs (sync + scalar HW queues in compute order).

Per-core dataflow (bf16 matmuls, fp32 PSUM accumulate / softmax math):
  XT slab [128, qc*2048 + c*512 + s] resident in SBUF
  QT/KT per head-pair per q-half: [128 dd, 512 q] (W chunk stationary);
      all QTs emitted before all KTs so the PE never queues behind the WK
      DMA; Q/K bias applied on ScalarE (Identity + per-partition bias AP)
  V = (X @ Wv + bv) -> 8 tiles [128 s, 8*(64+1)] with a ones column per head
      so the AV matmul also emits softmax denominators
  attention per q-chunk as ONE flat block stream across all 4 head-pairs,
  software-pipelined one k-block ahead (the pipeline crosses pair
  boundaries, so the PE never drains at a pair switch):
    score^T blocks [128 k, 2x512 q] on PE (k-blocks above the diagonal
    skipped, partial blocks column-trimmed), causal triangle fixed up on the
    PE itself (-1e9*I @ tri01), exp on ScalarE (scale=1/8 folded),
    AV accum O^T[65, q] with denom row 64.
  finalize per pair: copy denom row PSUM->SBUF (ScalarE for the last
    processed pair of each phase, DVE otherwise), reciprocal_approx_fast on
    DVE, GpSimd partition-broadcast, DVE scale; head-pair upper half placed
    via SBUF->SBUF DMA -- except the LAST PROCESSED pair of phase 1
    (head-pair 2; phase-1 pairs run in order 3,0,1,2), whose projection
    reads the scaled halves directly via two K=64 matmuls, so no DMA sits
    on the tail-critical path.
  Y[s-chunk] = sum_p O_pair^T-block^T @ Wo: si 0-3 emitted during the
  interphase (overlapping qc=1 attention), si 4-7 at the tail with
  pair-0/1/3 contributions emitted ahead of the finalize-gated pair-2
  splits; yp4/5 reuse the score-pipeline PSUM slots. The y copy applies
  +bo via scalar_tensor_tensor on DVE, and bf16 y stores stream out on
  both the sync and scalar HW queues; the host converts back to fp32.
"""

import numpy as np
import ml_dtypes

import concourse.bass as bass
import concourse.tile as tile
import concourse.mybir as mybir
from concourse import bacc
from concourse import bass_utils

B, S, E = 8, 1024, 512
H, DK, DV = 8, 64, 64
HD = H * DK  # 512
P = 128
EC = E // P  # 4 contraction chunks over E
NPAIR = H // 2
NCORES = 8
F32 = mybir.dt.float32
BF16 = mybir.dt.bfloat16
NEG = -1.0e9

_COMPILED = None

# phase-1 pair processing order: pair 2 last (it feeds the tail-path splits)
P1_ORDER = (3, 0, 1, 2)
TAIL_PAIR = P1_ORDER[-1]


def _body(nc, tc, const, work, ps, pb, d):
    # ---- const tiles + packed input DMAs ----
    XT = const.tile([P, 4096], BF16, tag="xt", name="XT")
    WQ = const.tile([P, 2048], BF16, tag="wq", name="WQ")   # (pair, c, dd)
    WK = const.tile([P, 2048], BF16, tag="wk", name="WK")   # (pair, c, dd)
    WV = const.tile([P, 2048], BF16, tag="wv", name="WV")   # (c, hd)
    WO = const.tile([P, 2048], BF16, tag="wo", name="WO")   # (c, e)
    CB = const.tile([P, 2176], BF16, tag="cb", name="CB")
    FB = const.tile([P, 8], F32, tag="fb", name="FB")

    # sync queue carries the compute-gating chain; scalar queue the rest
    nc.sync.dma_start(WQ[:, 0:1024], d["wq"][:, 0:1024])
    nc.sync.dma_start(XT[:, 0:1024], d["xt"][:, 0:1024])
    nc.sync.dma_start(XT[:, 1024:2048], d["xt"][:, 1024:2048])
    nc.sync.dma_start(WK[:, 0:1024], d["wk"][:, 0:1024])
    nc.sync.dma_start(WV[:], d["wv"][:])
    nc.sync.dma_start(XT[:, 2048:4096], d["xt"][:, 2048:4096])
    nc.sync.dma_start(WO[:], d["wo"][:])
    nc.scalar.dma_start(FB[:], d["fb"][:])
    nc.scalar.dma_start(WQ[:, 1024:2048], d["wq"][:, 1024:2048])
    nc.scalar.dma_start(WK[:, 1024:2048], d["wk"][:, 1024:2048])
    nc.scalar.dma_start(CB[:], d["cb"][:])

    bq_t = FB[:, 0:4]
    bk_t = FB[:, 4:8]
    negi = CB[:, 0:128]
    tri2 = CB[:, 128:384]
    bvb = CB[:, 384:896]
    bob = CB[:, 896:1408]
    # Wo rows for the tail pair's upper head, re-homed at partition base 0
    wo3b = CB[0:DV, 1408:1920]
    # keep-mask (k <= q), duplicated for both heads of a pair
    keep2 = CB[:, 1920:2176]

    qt, kt, ot_sb = {}, {}, {}
    ot_tmp = None
    v_sb = [None] * 8

    def emit_qk_one(p, qc, which):
        W, bias, store, nm = ((WQ, bq_t, qt, "q") if which == "q"
                              else (WK, bk_t, kt, "k"))
        pp = ps.tile([P, 512], F32, tag="ps", name=f"{nm}p{p}{qc}")
        for c in range(EC):
            nc.tensor.matmul(
                pp[:], W[:, c * 512 + p * P:c * 512 + (p + 1) * P],
                XT[:, qc * 2048 + c * 512:qc * 2048 + (c + 1) * 512],
                start=(c == 0), stop=(c == EC - 1))
        t = const.tile([P, 512], BF16, tag=f"{nm}t{p}{qc}", name=f"{nm}t{p}{qc}")
        nc.scalar.activation(
            t[:], pp[:], mybir.ActivationFunctionType.Identity,
            bias=bias[:, p:p + 1])
        store[p, qc] = t

    def emit_qkt(p, qc):
        emit_qk_one(p, qc, "q")
        emit_qk_one(p, qc, "k")

    def emit_v(si):
        qc, sl = divmod(si, 4)
        vp = ps.tile([P, HD], F32, tag="ps", name=f"vp{si}")
        for c in range(EC):
            base = qc * 2048 + c * 512
            nc.tensor.matmul(
                vp[:], XT[:, base + sl * P:base + (sl + 1) * P],
                WV[:, c * 512:(c + 1) * 512],
                start=(c == 0), stop=(c == EC - 1))
        t = const.tile([P, H * 65], BF16, tag=f"v{si}", name=f"v{si}")
        t3 = t.rearrange("p (h c) -> p h c", c=65)
        nc.gpsimd.memset(t[:], 1.0)  # contiguous; leaves the per-head ones column
        nc.vector.tensor_add(
            t3[:, :, 0:DV],
            vp.rearrange("p (h c) -> p h c", c=DV),
            bvb.rearrange("p (h c) -> p h c", c=DV))
        v_sb[si] = t

    def emit_y(si, yp):
        yo = work.tile([P, E], BF16, tag="yo", name=f"yo{si}", bufs=4)
        nc.vector.scalar_tensor_tensor(yo[:], yp[:], 1.0, bob[:],
                                       mybir.AluOpType.mult, mybir.AluOpType.add)
        q = nc.sync if si % 2 == 0 else nc.scalar
        q.dma_start(d["y"][si * P:(si + 1) * P, :], yo[:])

    def emit_proj(si):
        qc, sl = divmod(si, 4)
        yp = ps.tile([P, E], F32, tag="ps", name=f"yp{si}")
        for p in range(NPAIR):
            nc.tensor.matmul(
                yp[:], ot_sb[p, qc][:, sl * P:(sl + 1) * P],
                WO[:, p * 512:(p + 1) * 512],
                start=(p == 0), stop=(p == NPAIR - 1), skip_group_check=True)
        emit_y(si, yp)

    def emit_proj_tail():
        # si 4..7: pairs 0/1/3 emitted first (they overlap the last-processed
        # pair-2 finalize chain), then pair 2 via two K=64 matmuls straight
        # from the scaled halves (no SBUF->SBUF DMA on the tail path).
        yps = {}
        for si in range(4, 8):
            pool, tag = (pb, "st") if si < 6 else (ps, "ps")
            yps[si] = pool.tile([P, E], F32, tag=tag, name=f"yp{si}")

        def part(si, p, start):  # one non-tail pair contribution, K=128
            sl = si - 4
            nc.tensor.matmul(
                yps[si][:], ot_sb[p, 1][:, sl * P:(sl + 1) * P],
                WO[:, p * 512:(p + 1) * 512],
                start=start, stop=False, skip_group_check=True)

        def split(si, hb):  # tail-pair halves, K=64 from the scaled temps
            sl = si - 4
            tp = TAIL_PAIR
            if hb == 0:
                nc.tensor.matmul(
                    yps[si][:], ot_sb[tp, 1][0:DV, sl * P:(sl + 1) * P],
                    WO[0:DV, tp * 512:(tp + 1) * 512],
                    start=False, stop=False, tile_position=(0, 0),
                    skip_group_check=True)
            else:
                nc.tensor.matmul(
                    yps[si][:], ot_tmp[:, sl * P:(sl + 1) * P],
                    wo3b[:],
                    start=False, stop=True, tile_position=(0, 0),
                    skip_group_check=True)

        others = [p for p in range(NPAIR) if p != TAIL_PAIR]
        for si in (4, 5, 6, 7):
            for i, p in enumerate(others):
                part(si, p, i == 0)
        for si_pair in ((4, 5), (6, 7)):
            for si in si_pair:
                split(si, 0)
            for si in si_pair:
                split(si, 1)
        for si in range(4, 8):
            emit_y(si, yps[si])

    otps = {}

    def finalize(p, qc, act_copy=False):
        # ---- O^T *= 1/denom (reciprocal needs an SBUF input on HW, so the
        # PSUM denom row is copied first -- on ACT for the last pair of a
        # phase, where ACT is about to idle and the chain gates psum reuse)
        otp = otps[p, qc]
        tail_pair = (qc == 1 and p == TAIL_PAIR)
        ot = const.tile([P, 512], BF16, tag=f"ot{p}{qc}", name=f"ot{p}{qc}")
        rb = {}
        for hb in (0, 1):
            rrow = work.tile([1, 512], F32, tag="rrow", name=f"rrow{p}{qc}{hb}",
                             bufs=2)
            if act_copy:
                nc.scalar.activation(rrow[:], otp[hb][DV:DV + 1, :],
                                     mybir.ActivationFunctionType.Copy)
            else:
                nc.vector.tensor_copy(rrow[:], otp[hb][DV:DV + 1, :])
            rec = work.tile([1, 512], F32, tag="rec", name=f"rec{p}{qc}{hb}",
                            bufs=2)
            nc.vector.reciprocal_approx_fast(rec[:], rrow[:])
            rb[hb] = work.tile([DV, 512], F32, tag="rb", name=f"rb{p}{qc}{hb}",
                               bufs=2)
            nc.gpsimd.partition_broadcast(rb[hb][:], rec[:])
        nc.vector.tensor_mul(ot[0:DV, :], otp[0][0:DV, :], rb[0][:])
        # DVE cannot shift partitions: scale into a temp at base 0, then
        # SBUF->SBUF DMA into partitions 64-127 of the pair tile (skipped
        # for the tail pair, whose projection reads the temp directly)
        tmp = work.tile([DV, 512], BF16, tag="ottmp", name=f"ottmp{p}{qc}",
                        bufs=2)
        nc.vector.tensor_mul(tmp[:], otp[1][0:DV, :], rb[1][:])
        if tail_pair:
            nonlocal ot_tmp
            ot_tmp = tmp
        else:
            nc.sync.dma_start(ot[DV:P, :], tmp[:])
        ot_sb[p, qc] = ot

    def attn_phase(qc):
        # flat block stream across all pairs with a TWO-block AV lookahead:
        # AV(i) issues after scores(i+1), scores(i+2) are queued, so the PE
        # has ~2 score blocks of runway over the exp latency
        n_ki = 4 * (qc + 1)
        order = P1_ORDER if qc == 1 else tuple(range(NPAIR))
        blocks = [(p, ki) for p in order for ki in range(n_ki)]
        stps, stes = {}, {}

        def emit_score(p, ki):
            kc, kl = divmod(ki, 4)
            diag = (ki * P - qc * 512) >= 0
            off = max(ki * P - qc * 512, 0)
            # qc=1 diag blocks are masked post-exp on DVE instead; qc=0 keeps
            # the PE fixup since the DVE would gate the exp pipeline
            pe_fix = diag and qc == 0
            stp = pb.tile([P, 1024], F32, tag="st", name=f"st{p}{qc}{ki}")
            for hb in (0, 1):
                hp = slice(hb * DK, (hb + 1) * DK)
                nc.tensor.matmul(
                    stp[:, hb * 512 + off:(hb + 1) * 512],
                    kt[p, kc][hp, kl * P:(kl + 1) * P],
                    qt[p, qc][hp, off:],
                    start=True, stop=not pe_fix, tile_position=(hb * DK, 0),
                    skip_group_check=True)
            if pe_fix:
                for hb in (0, 1):
                    nc.tensor.matmul(
                        stp[:, hb * 512 + off:hb * 512 + off + P],
                        negi[:], tri2[:, 0:P],
                        start=False, stop=True, skip_group_check=True)
            stps[p, ki] = (stp, off, diag and qc == 1)

        def emit_exp(p, ki):
            stp, off, dve_mask = stps[p, ki]
            ste = work.tile([P, 1024], BF16, tag="ste", name=f"ste{p}{qc}{ki}",
                            bufs=3)
            if off == 0:
                nc.scalar.activation(
                    ste[:], stp[:], mybir.ActivationFunctionType.Exp,
                    scale=0.125)
            else:
                stp3 = stp.rearrange("p (h q) -> p h q", h=2)[:, :, off:]
                ste3 = ste.rearrange("p (h q) -> p h q", h=2)[:, :, off:]
                nc.scalar.activation(
                    ste3, stp3, mybir.ActivationFunctionType.Exp, scale=0.125)
            if dve_mask:
                # zero exp'd scores above the diagonal (both heads at once)
                sv = ste.rearrange("p (h q) -> p h q", h=2)[:, :, off:off + P]
                nc.vector.tensor_mul(
                    sv, sv, keep2.rearrange("p (h q) -> p h q", h=2))
            stes[p, ki] = (ste, off)

        def emit_av(p, ki):
            ste, off = stes[p, ki]
            st_f, sp_f = (ki == 0), (ki == n_ki - 1)
            for hb in (0, 1):
                h = 2 * p + hb
                nc.tensor.matmul(
                    otps[p, qc][hb][:, off:], v_sb[ki][:, h * 65:h * 65 + 65],
                    ste[:, hb * 512 + off:(hb + 1) * 512],
                    start=st_f, stop=sp_f, skip_group_check=True)

        for idx, (p, ki) in enumerate(blocks):
            if ki == 0:
                otps[p, qc] = [ps.tile([DV + 1, 512], F32, tag="ps",
                                       name=f"otp{p}{qc}{hb}") for hb in (0, 1)]
            emit_score(p, ki)
            emit_exp(p, ki)
            if idx >= 1:
                pp, pk = blocks[idx - 1]
                emit_av(pp, pk)
                if pk == n_ki - 1:
                    finalize(pp, qc, act_copy=(pp == order[-1]))
        emit_av(*blocks[-1])
        finalize(order[-1], qc, act_copy=True)

    # ---- emission schedule (ps-tag rotation keeps otp pairs ping-ponging
    # between bank pairs) ----
    for p in range(NPAIR):            # ps allocs 0-7; all QTs first so the
        emit_qk_one(p, 0, "q")        # PE never queues behind the WK DMA
    for p in range(NPAIR):
        emit_qk_one(p, 0, "k")
    for si in range(4):               # 8-11
        emit_v(si)
    attn_phase(0)                     # 12-19 -> slots (0,1)/(2,3)/(0,1)/(2,3)
    emit_qkt(0, 1)                    # 20-21
    emit_qkt(1, 1)                    # 22-23
    emit_v(4)                         # 24
    emit_v(5)                         # 25
    emit_qkt(2, 1)                    # 26-27
    emit_qkt(3, 1)                    # 28-29
    emit_v(6)                         # 30
    emit_v(7)                         # 31
    for si in range(4):               # (needs all qc=0 ot, finalized)
        emit_proj(si)
    attn_phase(1)                     # 36-43 -> slots (0,1)/(2,3)/(0,1)/(2,3)
    emit_proj_tail()                  # yp6/7 at ps 44,45 -> slots 0,1


def _build():
    nc = bacc.Bacc("TRN2", target_bir_lowering=False, debug=False)
    d = {
        "xt": nc.dram_tensor("xt", [P, 4096], BF16, kind="ExternalInput").ap(),
        "wq": nc.dram_tensor("wq", [P, 2048], BF16, kind="ExternalInput").ap(),
        "wk": nc.dram_tensor("wk", [P, 2048], BF16, kind="ExternalInput").ap(),
        "wv": nc.dram_tensor("wv", [P, 2048], BF16, kind="ExternalInput").ap(),
        "wo": nc.dram_tensor("wo", [P, 2048], BF16, kind="ExternalInput").ap(),
        "cb": nc.dram_tensor("cb", [P, 2176], BF16, kind="ExternalInput").ap(),
        "fb": nc.dram_tensor("fb", [P, 8], F32, kind="ExternalInput").ap(),
        "y": nc.dram_tensor("y", [S, E], BF16, kind="ExternalOutput").ap(),
    }
    with tile.TileContext(nc) as tc:
        with tc.tile_pool(name="const", bufs=1) as const, \
             tc.tile_pool(name="work", bufs=3) as work, \
             tc.tile_pool(name="ps", bufs=4, space="PSUM") as ps, \
             tc.tile_pool(name="pb", bufs=2, space="PSUM") as pb:
            _body(nc, tc, const, work, ps, pb, d)
    nc.compile()
    return nc


def get_nc():
    global _COMPILED
    if _COMPILED is None:
        _COMPILED = _build()
    return _COMPILED


def _prep_in_maps(X, Wq, bq, Wk, bk, Wv, bv, Wo, bo):
    f = np.float32
    bf = ml_dtypes.bfloat16

    def wslab(W):  # [H,E,Dk] -> [128, c*512 + (h*64+d)]
        Wr = np.transpose(np.asarray(W, f), (1, 0, 2)).reshape(E, HD)
        return np.ascontiguousarray(
            Wr.reshape(EC, P, HD).transpose(1, 0, 2).reshape(P, EC * HD).astype(bf))

    shared = {
        "wq": wslab(Wq),
        "wk": wslab(Wk),
        "wv": wslab(Wv),
        "wo": np.ascontiguousarray(
            np.asarray(Wo, f).reshape(EC, P, E).transpose(1, 0, 2)
            .reshape(P, EC * E).astype(bf)),
    }
    bq_t = np.asarray(bq, f).reshape(HD).reshape(NPAIR, P).T
    bk_t = np.asarray(bk, f).reshape(HD).reshape(NPAIR, P).T
    bvb = np.broadcast_to(np.asarray(bv, f).reshape(1, HD), (P, HD)).astype(bf)
    bob = np.broadcast_to(np.asarray(bo, f).reshape(1, E), (P, E)).astype(bf)
    kk = np.arange(P)[:, None]
    jj = np.arange(P)[None, :]
    shared["fb"] = np.ascontiguousarray(
        np.concatenate([bq_t, bk_t], axis=1).astype(f))
    negi = (np.eye(P, dtype=f) * NEG).astype(bf)
    tri01 = (kk > jj).astype(bf)
    # Wo rows for the tail pair's upper head, re-homed to partitions 0:64
    wo3b = np.zeros((P, E), dtype=bf)
    wo3b[0:DV] = np.asarray(Wo, f)[(2 * TAIL_PAIR + 1) * DV:
                                   (2 * TAIL_PAIR + 2) * DV, :].astype(bf)
    keep01 = (kk <= jj).astype(bf)
    shared["cb"] = np.ascontiguousarray(
        np.concatenate([negi, tri01, tri01, bvb, bob, wo3b, keep01, keep01],
                       axis=1))

    Xf = np.asarray(X, f)
    in_maps = []
    for b in range(B):
        m = dict(shared)
        # xt slab: [128, qc*2048 + c*512 + s']
        m["xt"] = np.ascontiguousarray(
            Xf[b].T.reshape(EC, P, 2, 512).transpose(1, 2, 0, 3)
            .reshape(P, 4096).astype(bf))
        in_maps.append(m)
    return in_maps


def kernel(X, Wq, bq, Wk, bk, Wv, bv, Wo, bo):
    nc = get_nc()
    in_maps = _prep_in_maps(X, Wq, bq, Wk, bk, Wv, bv, Wo, bo)
    res = bass_utils.run_bass_kernel_spmd(nc, in_maps, core_ids=list(range(NCORES)))
    return np.stack([res.results[b]["y"] for b in range(B)], axis=0).astype(np.float32)


def run_traced(X, Wq, bq, Wk, bk, Wv, bv, Wo, bo):
    """Like kernel() but with NTFF profiling; returns (out, exec_time_ns)."""
    nc = get_nc()
    in_maps = _prep_in_maps(X, Wq, bq, Wk, bk, Wv, bv, Wo, bo)
    res = bass_utils.run_bass_kernel_spmd(
        nc, in_maps, core_ids=list(range(NCORES)), trace=True)
    out = np.stack([res.results[b]["y"] for b in range(B)], axis=0).astype(np.float32)
    return out, res.exec_time_ns


# revision 24
# speedup vs baseline: 1.0126x; 1.0049x over previous
"""Multi-head causal attention (B=8, S=1024, E=512, H=8, Dk=Dv=64) on 8 NeuronCores.

Sharding: data-parallel over batch. Core b computes the full attention block
for X[b]; no collectives. Host pre-transposes X[b] -> [E, S], converts matmul
operands to bf16, and packs weights into wide slabs so the device issues only
a few input DMAs (sync + scalar HW queues in compute order).

Per-core dataflow (bf16 matmuls, fp32 PSUM accumulate / softmax math):
  XT slab [128, qc*2048 + c*512 + s] resident in SBUF
  QT/KT per head-pair per q-half: [128 dd, 512 q] (W chunk stationary);
      all QTs emitted before all KTs so the PE never queues behind the WK
      DMA; Q/K bias applied on ScalarE (Identity + per-partition bias AP)
  V = (X @ Wv + bv) -> 8 tiles [128 s, 8*(64+1)] with a ones column per head
      so the AV matmul also emits softmax denominators
  attention per q-chunk as ONE flat block stream across all 4 head-pairs,
  software-pipelined one k-block ahead (the pipeline crosses pair
  boundaries, so the PE never drains at a pair switch):
    score^T blocks [128 k, 2x512 q] on PE (k-blocks above the diagonal
    skipped, partial blocks column-trimmed), causal triangle fixed up on the
    PE itself (-1e9*I @ tri01), exp on ScalarE (scale=1/8 folded),
    AV accum O^T[65, q] with denom row 64.
  finalize per pair: copy denom row PSUM->SBUF (ScalarE for the last
    processed pair of each phase, DVE otherwise), reciprocal_approx_fast on
    DVE, GpSimd partition-broadcast, DVE scale; head-pair upper half placed
    via SBUF->SBUF DMA -- except the LAST PROCESSED pair of phase 1
    (head-pair 2; phase-1 pairs run in order 3,0,1,2), whose projection
    reads the scaled halves directly via two K=64 matmuls, so no DMA sits
    on the tail-critical path.
  Y[s-chunk] = sum_p O_pair^T-block^T @ Wo: si 0-3 emitted during the
  interphase (overlapping qc=1 attention), si 4-7 at the tail with
  pair-0/1/3 contributions emitted ahead of the finalize-gated pair-2
  splits; yp4/5 reuse the score-pipeline PSUM slots. The y copy applies
  +bo via scalar_tensor_tensor on DVE, and bf16 y stores stream out on
  both the sync and scalar HW queues; the host converts back to fp32.
"""

import numpy as np
import ml_dtypes

import concourse.bass as bass
import concourse.tile as tile
import concourse.mybir as mybir
from concourse import bacc
from concourse import bass_utils

B, S, E = 8, 1024, 512
H, DK, DV = 8, 64, 64
HD = H * DK  # 512
P = 128
EC = E // P  # 4 contraction chunks over E
NPAIR = H // 2
NCORES = 8
F32 = mybir.dt.float32
BF16 = mybir.dt.bfloat16
NEG = -1.0e9

_COMPILED = None

# phase-1 pair processing order: pair 2 last (it feeds the tail-path splits)
P1_ORDER = (3, 0, 1, 2)
TAIL_PAIR = P1_ORDER[-1]


def _body(nc, tc, const, work, ps, pb, d):
    # ---- const tiles + packed input DMAs ----
    XT = const.tile([P, 4096], BF16, tag="xt", name="XT")
    WQ = const.tile([P, 2048], BF16, tag="wq", name="WQ")   # (pair, c, dd)
    WK = const.tile([P, 2048], BF16, tag="wk", name="WK")   # (pair, c, dd)
    WV = const.tile([P, 2048], BF16, tag="wv", name="WV")   # (c, hd)
    WO = const.tile([P, 2048], BF16, tag="wo", name="WO")   # (c, e)
    CB = const.tile([P, 2176], BF16, tag="cb", name="CB")
    FB = const.tile([P, 8], F32, tag="fb", name="FB")

    # sync queue carries the compute-gating chain; scalar queue the rest
    nc.sync.dma_start(WQ[:, 0:1024], d["wq"][:, 0:1024])
    nc.sync.dma_start(XT[:, 0:1024], d["xt"][:, 0:1024])
    nc.sync.dma_start(XT[:, 1024:2048], d["xt"][:, 1024:2048])
    nc.sync.dma_start(WK[:, 0:1024], d["wk"][:, 0:1024])
    nc.sync.dma_start(WV[:], d["wv"][:])
    nc.sync.dma_start(XT[:, 2048:4096], d["xt"][:, 2048:4096])
    nc.sync.dma_start(WO[:], d["wo"][:])
    nc.scalar.dma_start(FB[:], d["fb"][:])
    nc.scalar.dma_start(WQ[:, 1024:2048], d["wq"][:, 1024:2048])
    nc.scalar.dma_start(WK[:, 1024:2048], d["wk"][:, 1024:2048])
    nc.scalar.dma_start(CB[:], d["cb"][:])

    bq_t = FB[:, 0:4]
    bk_t = FB[:, 4:8]
    negi = CB[:, 0:128]
    tri2 = CB[:, 128:384]
    bvb = CB[:, 384:896]
    bob = CB[:, 896:1408]
    # Wo rows for the tail pair's upper head, re-homed at partition base 0
    wo3b = CB[0:DV, 1408:1920]
    # keep-mask (k <= q), duplicated for both heads of a pair
    keep2 = CB[:, 1920:2176]

    qt, kt, ot_sb = {}, {}, {}
    ot_tmp = None
    v_sb = [None] * 8

    def emit_qk_one(p, qc, which):
        W, bias, store, nm = ((WQ, bq_t, qt, "q") if which == "q"
                              else (WK, bk_t, kt, "k"))
        pp = ps.tile([P, 512], F32, tag="ps", name=f"{nm}p{p}{qc}")
        for c in range(EC):
            nc.tensor.matmul(
                pp[:], W[:, c * 512 + p * P:c * 512 + (p + 1) * P],
                XT[:, qc * 2048 + c * 512:qc * 2048 + (c + 1) * 512],
                start=(c == 0), stop=(c == EC - 1))
        t = const.tile([P, 512], BF16, tag=f"{nm}t{p}{qc}", name=f"{nm}t{p}{qc}")
        nc.scalar.activation(
            t[:], pp[:], mybir.ActivationFunctionType.Identity,
            bias=bias[:, p:p + 1])
        store[p, qc] = t

    def emit_qkt(p, qc):
        emit_qk_one(p, qc, "q")
        emit_qk_one(p, qc, "k")

    def emit_v(si):
        qc, sl = divmod(si, 4)
        vp = ps.tile([P, HD], F32, tag="ps", name=f"vp{si}")
        for c in range(EC):
            base = qc * 2048 + c * 512
            nc.tensor.matmul(
                vp[:], XT[:, base + sl * P:base + (sl + 1) * P],
                WV[:, c * 512:(c + 1) * 512],
                start=(c == 0), stop=(c == EC - 1))
        t = const.tile([P, H * 65], BF16, tag=f"v{si}", name=f"v{si}")
        t3 = t.rearrange("p (h c) -> p h c", c=65)
        nc.gpsimd.memset(t[:], 1.0)  # contiguous; leaves the per-head ones column
        nc.vector.tensor_add(
            t3[:, :, 0:DV],
            vp.rearrange("p (h c) -> p h c", c=DV),
            bvb.rearrange("p (h c) -> p h c", c=DV))
        v_sb[si] = t

    def emit_y(si, yp):
        yo = work.tile([P, E], BF16, tag="yo", name=f"yo{si}", bufs=4)
        nc.vector.scalar_tensor_tensor(yo[:], yp[:], 1.0, bob[:],
                                       mybir.AluOpType.mult, mybir.AluOpType.add)
        q = nc.sync if si % 2 == 0 else nc.scalar
        q.dma_start(d["y"][si * P:(si + 1) * P, :], yo[:])

    def emit_proj(si):
        qc, sl = divmod(si, 4)
        yp = ps.tile([P, E], F32, tag="ps", name=f"yp{si}")
        for p in range(NPAIR):
            nc.tensor.matmul(
                yp[:], ot_sb[p, qc][:, sl * P:(sl + 1) * P],
                WO[:, p * 512:(p + 1) * 512],
                start=(p == 0), stop=(p == NPAIR - 1), skip_group_check=True)
        emit_y(si, yp)

    def emit_proj_tail():
        # si 4..7: pairs 0/1/3 emitted first (they overlap the last-processed
        # pair-2 finalize chain), then pair 2 via two K=64 matmuls straight
        # from the scaled halves (no SBUF->SBUF DMA on the tail path).
        yps = {}
        for si in range(4, 8):
            pool, tag = (pb, "st") if si < 6 else (ps, "ps")
            yps[si] = pool.tile([P, E], F32, tag=tag, name=f"yp{si}")

        def part(si, p, start):  # one non-tail pair contribution, K=128
            sl = si - 4
            nc.tensor.matmul(
                yps[si][:], ot_sb[p, 1][:, sl * P:(sl + 1) * P],
                WO[:, p * 512:(p + 1) * 512],
                start=start, stop=False, skip_group_check=True)

        def split(si, hb):  # tail-pair halves, K=64 from the scaled temps
            sl = si - 4
            tp = TAIL_PAIR
            if hb == 0:
                nc.tensor.matmul(
                    yps[si][:], ot_sb[tp, 1][0:DV, sl * P:(sl + 1) * P],
                    WO[0:DV, tp * 512:(tp + 1) * 512],
                    start=False, stop=False, tile_position=(0, 0),
                    skip_group_check=True)
            else:
                nc.tensor.matmul(
                    yps[si][:], ot_tmp[:, sl * P:(sl + 1) * P],
                    wo3b[:],
                    start=False, stop=True, tile_position=(0, 0),
                    skip_group_check=True)

        others = [p for p in range(NPAIR) if p != TAIL_PAIR]
        for si in (4, 5, 6, 7):
            for i, p in enumerate(others):
                part(si, p, i == 0)
        for si_pair in ((4, 5), (6, 7)):
            for si in si_pair:
                split(si, 0)
            for si in si_pair:
                split(si, 1)
        for si in range(4, 8):
            emit_y(si, yps[si])

    otps = {}

    def finalize(p, qc, act_copy=False):
        # ---- O^T *= 1/denom (reciprocal needs an SBUF input on HW, so the
        # PSUM denom row is copied first -- on ACT for the last pair of a
        # phase, where ACT is about to idle and the chain gates psum reuse)
        otp = otps[p, qc]
        tail_pair = (qc == 1 and p == TAIL_PAIR)
        ot = const.tile([P, 512], BF16, tag=f"ot{p}{qc}", name=f"ot{p}{qc}")
        tmp = work.tile([DV, 512], BF16, tag="ottmp", name=f"ottmp{p}{qc}",
                        bufs=2)
        dst = {0: ot[0:DV, :], 1: tmp[:]}
        if act_copy:
            # last processed pair: this chain gates psum slot reuse (and the
            # tail splits), so run every stage in half-width pieces that
            # pipeline across ACT -> DVE -> GpSimd -> DVE
            rrow, rec, rb = {}, {}, {}
            for hb in (0, 1):
                rrow[hb] = work.tile([1, 512], F32, tag="rrow",
                                     name=f"rrow{p}{qc}{hb}", bufs=2)
                rec[hb] = work.tile([1, 512], F32, tag="rec",
                                    name=f"rec{p}{qc}{hb}", bufs=2)
                rb[hb] = work.tile([DV, 512], F32, tag="rb",
                                   name=f"rb{p}{qc}{hb}", bufs=2)
            halves = [(hb, slice(h * 256, (h + 1) * 256))
                      for hb in (0, 1) for h in (0, 1)]
            for hb, sl in halves:
                nc.scalar.activation(rrow[hb][:, sl], otp[hb][DV:DV + 1, sl],
                                     mybir.ActivationFunctionType.Copy)
            for hb, sl in halves:
                nc.vector.reciprocal_approx_fast(rec[hb][:, sl],
                                                 rrow[hb][:, sl])
            for hb, sl in halves:
                nc.gpsimd.partition_broadcast(rb[hb][:, sl], rec[hb][:, sl])
            for hb, sl in halves:
                nc.vector.tensor_mul(dst[hb][:, sl], otp[hb][0:DV, sl],
                                     rb[hb][:, sl])
        else:
            rb = {}
            for hb in (0, 1):
                rrow = work.tile([1, 512], F32, tag="rrow",
                                 name=f"rrow{p}{qc}{hb}", bufs=2)
                nc.vector.tensor_copy(rrow[:], otp[hb][DV:DV + 1, :])
                rec = work.tile([1, 512], F32, tag="rec",
                                name=f"rec{p}{qc}{hb}", bufs=2)
                nc.vector.reciprocal_approx_fast(rec[:], rrow[:])
                rb[hb] = work.tile([DV, 512], F32, tag="rb",
                                   name=f"rb{p}{qc}{hb}", bufs=2)
                nc.gpsimd.partition_broadcast(rb[hb][:], rec[:])
            nc.vector.tensor_mul(ot[0:DV, :], otp[0][0:DV, :], rb[0][:])
            nc.vector.tensor_mul(tmp[:], otp[1][0:DV, :], rb[1][:])
        # DVE cannot shift partitions: the upper head is scaled into a temp at
        # base 0, then SBUF->SBUF DMA'd into partitions 64-127 of the pair
        # tile (skipped for the tail pair: its projection reads the temp)
        if tail_pair:
            nonlocal ot_tmp
            ot_tmp = tmp
        else:
            nc.sync.dma_start(ot[DV:P, :], tmp[:])
        ot_sb[p, qc] = ot

    def attn_phase(qc):
        # flat block stream across all pairs with a TWO-block AV lookahead:
        # AV(i) issues after scores(i+1), scores(i+2) are queued, so the PE
        # has ~2 score blocks of runway over the exp latency
        n_ki = 4 * (qc + 1)
        order = P1_ORDER if qc == 1 else tuple(range(NPAIR))
        blocks = [(p, ki) for p in order for ki in range(n_ki)]
        stps, stes = {}, {}

        def emit_score(p, ki):
            kc, kl = divmod(ki, 4)
            diag = (ki * P - qc * 512) >= 0
            off = max(ki * P - qc * 512, 0)
            # qc=1 diag blocks are masked post-exp on DVE instead; qc=0 keeps
            # the PE fixup since the DVE would gate the exp pipeline
            pe_fix = diag and qc == 0
            stp = pb.tile([P, 1024], F32, tag="st", name=f"st{p}{qc}{ki}")
            for hb in (0, 1):
                hp = slice(hb * DK, (hb + 1) * DK)
                nc.tensor.matmul(
                    stp[:, hb * 512 + off:(hb + 1) * 512],
                    kt[p, kc][hp, kl * P:(kl + 1) * P],
                    qt[p, qc][hp, off:],
                    start=True, stop=not pe_fix, tile_position=(hb * DK, 0),
                    skip_group_check=True)
            if pe_fix:
                for hb in (0, 1):
                    nc.tensor.matmul(
                        stp[:, hb * 512 + off:hb * 512 + off + P],
                        negi[:], tri2[:, 0:P],
                        start=False, stop=True, skip_group_check=True)
            stps[p, ki] = (stp, off, diag and qc == 1)

        def emit_exp(p, ki):
            stp, off, dve_mask = stps[p, ki]
            ste = work.tile([P, 1024], BF16, tag="ste", name=f"ste{p}{qc}{ki}",
                            bufs=3)
            if off == 0:
                nc.scalar.activation(
                    ste[:], stp[:], mybir.ActivationFunctionType.Exp,
                    scale=0.125)
            else:
                stp3 = stp.rearrange("p (h q) -> p h q", h=2)[:, :, off:]
                ste3 = ste.rearrange("p (h q) -> p h q", h=2)[:, :, off:]
                nc.scalar.activation(
                    ste3, stp3, mybir.ActivationFunctionType.Exp, scale=0.125)
            if dve_mask:
                # zero exp'd scores above the diagonal (both heads at once)
                sv = ste.rearrange("p (h q) -> p h q", h=2)[:, :, off:off + P]
                nc.vector.tensor_mul(
                    sv, sv, keep2.rearrange("p (h q) -> p h q", h=2))
            stes[p, ki] = (ste, off)

        def emit_av(p, ki):
            ste, off = stes[p, ki]
            st_f, sp_f = (ki == 0), (ki == n_ki - 1)
            for hb in (0, 1):
                h = 2 * p + hb
                nc.tensor.matmul(
                    otps[p, qc][hb][:, off:], v_sb[ki][:, h * 65:h * 65 + 65],
                    ste[:, hb * 512 + off:(hb + 1) * 512],
                    start=st_f, stop=sp_f, skip_group_check=True)

        for idx, (p, ki) in enumerate(blocks):
            if ki == 0:
                otps[p, qc] = [ps.tile([DV + 1, 512], F32, tag="ps",
                                       name=f"otp{p}{qc}{hb}") for hb in (0, 1)]
            emit_score(p, ki)
            emit_exp(p, ki)
            if idx >= 1:
                pp, pk = blocks[idx - 1]
                emit_av(pp, pk)
                if pk == n_ki - 1:
                    finalize(pp, qc, act_copy=(pp == order[-1]))
        emit_av(*blocks[-1])
        finalize(order[-1], qc, act_copy=True)

    # ---- emission schedule (ps-tag rotation keeps otp pairs ping-ponging
    # between bank pairs) ----
    for p in range(NPAIR):            # ps allocs 0-7; all QTs first so the
        emit_qk_one(p, 0, "q")        # PE never queues behind the WK DMA
    for p in range(NPAIR):
        emit_qk_one(p, 0, "k")
    for si in range(4):               # 8-11
        emit_v(si)
    attn_phase(0)                     # 12-19 -> slots (0,1)/(2,3)/(0,1)/(2,3)
    emit_qkt(0, 1)                    # 20-21
    emit_qkt(1, 1)                    # 22-23
    emit_v(4)                         # 24
    emit_v(5)                         # 25
    emit_qkt(2, 1)                    # 26-27
    emit_qkt(3, 1)                    # 28-29
    emit_v(6)                         # 30
    emit_v(7)                         # 31
    for si in range(4):               # (needs all qc=0 ot, finalized)
        emit_proj(si)
    attn_phase(1)                     # 36-43 -> slots (0,1)/(2,3)/(0,1)/(2,3)
    emit_proj_tail()                  # yp6/7 at ps 44,45 -> slots 0,1


def _build():
    nc = bacc.Bacc("TRN2", target_bir_lowering=False, debug=False)
    d = {
        "xt": nc.dram_tensor("xt", [P, 4096], BF16, kind="ExternalInput").ap(),
        "wq": nc.dram_tensor("wq", [P, 2048], BF16, kind="ExternalInput").ap(),
        "wk": nc.dram_tensor("wk", [P, 2048], BF16, kind="ExternalInput").ap(),
        "wv": nc.dram_tensor("wv", [P, 2048], BF16, kind="ExternalInput").ap(),
        "wo": nc.dram_tensor("wo", [P, 2048], BF16, kind="ExternalInput").ap(),
        "cb": nc.dram_tensor("cb", [P, 2176], BF16, kind="ExternalInput").ap(),
        "fb": nc.dram_tensor("fb", [P, 8], F32, kind="ExternalInput").ap(),
        "y": nc.dram_tensor("y", [S, E], BF16, kind="ExternalOutput").ap(),
    }
    with tile.TileContext(nc) as tc:
        with tc.tile_pool(name="const", bufs=1) as const, \
             tc.tile_pool(name="work", bufs=3) as work, \
             tc.tile_pool(name="ps", bufs=4, space="PSUM") as ps, \
             tc.tile_pool(name="pb", bufs=2, space="PSUM") as pb:
            _body(nc, tc, const, work, ps, pb, d)
    nc.compile()
    return nc


def get_nc():
    global _COMPILED
    if _COMPILED is None:
        _COMPILED = _build()
    return _COMPILED


def _prep_in_maps(X, Wq, bq, Wk, bk, Wv, bv, Wo, bo):
    f = np.float32
    bf = ml_dtypes.bfloat16

    def wslab(W):  # [H,E,Dk] -> [128, c*512 + (h*64+d)]
        Wr = np.transpose(np.asarray(W, f), (1, 0, 2)).reshape(E, HD)
        return np.ascontiguousarray(
            Wr.reshape(EC, P, HD).transpose(1, 0, 2).reshape(P, EC * HD).astype(bf))

    shared = {
        "wq": wslab(Wq),
        "wk": wslab(Wk),
        "wv": wslab(Wv),
        "wo": np.ascontiguousarray(
            np.asarray(Wo, f).reshape(EC, P, E).transpose(1, 0, 2)
            .reshape(P, EC * E).astype(bf)),
    }
    bq_t = np.asarray(bq, f).reshape(HD).reshape(NPAIR, P).T
    bk_t = np.asarray(bk, f).reshape(HD).reshape(NPAIR, P).T
    bvb = np.broadcast_to(np.asarray(bv, f).reshape(1, HD), (P, HD)).astype(bf)
    bob = np.broadcast_to(np.asarray(bo, f).reshape(1, E), (P, E)).astype(bf)
    kk = np.arange(P)[:, None]
    jj = np.arange(P)[None, :]
    shared["fb"] = np.ascontiguousarray(
        np.concatenate([bq_t, bk_t], axis=1).astype(f))
    negi = (np.eye(P, dtype=f) * NEG).astype(bf)
    tri01 = (kk > jj).astype(bf)
    # Wo rows for the tail pair's upper head, re-homed to partitions 0:64
    wo3b = np.zeros((P, E), dtype=bf)
    wo3b[0:DV] = np.asarray(Wo, f)[(2 * TAIL_PAIR + 1) * DV:
                                   (2 * TAIL_PAIR + 2) * DV, :].astype(bf)
    keep01 = (kk <= jj).astype(bf)
    shared["cb"] = np.ascontiguousarray(
        np.concatenate([negi, tri01, tri01, bvb, bob, wo3b, keep01, keep01],
                       axis=1))

    Xf = np.asarray(X, f)
    in_maps = []
    for b in range(B):
        m = dict(shared)
        # xt slab: [128, qc*2048 + c*512 + s']
        m["xt"] = np.ascontiguousarray(
            Xf[b].T.reshape(EC, P, 2, 512).transpose(1, 2, 0, 3)
            .reshape(P, 4096).astype(bf))
        in_maps.append(m)
    return in_maps


def kernel(X, Wq, bq, Wk, bk, Wv, bv, Wo, bo):
    nc = get_nc()
    in_maps = _prep_in_maps(X, Wq, bq, Wk, bk, Wv, bv, Wo, bo)
    res = bass_utils.run_bass_kernel_spmd(nc, in_maps, core_ids=list(range(NCORES)))
    return np.stack([res.results[b]["y"] for b in range(B)], axis=0).astype(np.float32)


def run_traced(X, Wq, bq, Wk, bk, Wv, bv, Wo, bo):
    """Like kernel() but with NTFF profiling; returns (out, exec_time_ns)."""
    nc = get_nc()
    in_maps = _prep_in_maps(X, Wq, bq, Wk, bk, Wv, bv, Wo, bo)
    res = bass_utils.run_bass_kernel_spmd(
        nc, in_maps, core_ids=list(range(NCORES)), trace=True)
    out = np.stack([res.results[b]["y"] for b in range(B)], axis=0).astype(np.float32)
    return out, res.exec_time_ns


# revision 25
# speedup vs baseline: 1.0184x; 1.0057x over previous
"""Multi-head causal attention (B=8, S=1024, E=512, H=8, Dk=Dv=64) on 8 NeuronCores.

Sharding: data-parallel over batch. Core b computes the full attention block
for X[b]; no collectives. Host pre-transposes X[b] -> [E, S], converts matmul
operands to bf16, and packs weights into wide slabs so the device issues only
a few input DMAs (sync + scalar HW queues in compute order).

Per-core dataflow (bf16 matmuls, fp32 PSUM accumulate / softmax math):
  XT slab [128, qc*2048 + c*512 + s] resident in SBUF
  QT/KT per head-pair per q-half: [128 dd, 512 q] (W chunk stationary);
      all QTs emitted before all KTs so the PE never queues behind the WK
      DMA; Q/K bias applied on ScalarE (Identity + per-partition bias AP)
  V = (X @ Wv + bv) -> 8 tiles [128 s, 8*(64+1)] with a ones column per head
      so the AV matmul also emits softmax denominators
  attention per q-chunk as ONE flat block stream across all 4 head-pairs,
  software-pipelined one k-block ahead (the pipeline crosses pair
  boundaries, so the PE never drains at a pair switch):
    score^T blocks [128 k, 2x512 q] on PE (k-blocks above the diagonal
    skipped, partial blocks column-trimmed), causal triangle fixed up on the
    PE itself (-1e9*I @ tri01), exp on ScalarE (scale=1/8 folded),
    AV accum O^T[65, q] with denom row 64.
  finalize per pair: copy denom row PSUM->SBUF (ScalarE for the last
    processed pair of each phase, DVE otherwise), reciprocal_approx_fast on
    DVE, GpSimd partition-broadcast, DVE scale; the last processed pair runs
    every stage in half-width pieces so the ACT->DVE->GpSimd->DVE chain
    pipelines (it gates psum-slot reuse and the tail splits); head-pair
    upper half placed via SBUF->SBUF DMA -- except the LAST PROCESSED pair
    of phase 1
    (head-pair 2; phase-1 pairs run in order 3,0,1,2), whose projection
    reads the scaled halves directly via two K=64 matmuls, so no DMA sits
    on the tail-critical path.
  Y[s-chunk] = sum_p O_pair^T-block^T @ Wo: si 0-3 emitted during the
  interphase (overlapping qc=1 attention), si 4-7 at the tail with
  pair-0/1/3 contributions emitted ahead of the finalize-gated pair-2
  splits; yp4/5 reuse the score-pipeline PSUM slots. The y copy applies
  +bo via scalar_tensor_tensor on DVE, and bf16 y stores stream out on
  both the sync and scalar HW queues; the host converts back to fp32.
"""

import numpy as np
import ml_dtypes

import concourse.bass as bass
import concourse.tile as tile
import concourse.mybir as mybir
from concourse import bacc
from concourse import bass_utils

B, S, E = 8, 1024, 512
H, DK, DV = 8, 64, 64
HD = H * DK  # 512
P = 128
EC = E // P  # 4 contraction chunks over E
NPAIR = H // 2
NCORES = 8
F32 = mybir.dt.float32
BF16 = mybir.dt.bfloat16
NEG = -1.0e9

_COMPILED = None

# phase-1 pair processing order: pair 2 last (it feeds the tail-path splits)
P1_ORDER = (3, 0, 1, 2)
TAIL_PAIR = P1_ORDER[-1]


def _body(nc, tc, const, work, ps, pb, d):
    # ---- const tiles + packed input DMAs ----
    XT = const.tile([P, 4096], BF16, tag="xt", name="XT")
    WQ = const.tile([P, 2048], BF16, tag="wq", name="WQ")   # (pair, c, dd)
    WK = const.tile([P, 2048], BF16, tag="wk", name="WK")   # (pair, c, dd)
    WV = const.tile([P, 2048], BF16, tag="wv", name="WV")   # (c, hd)
    WO = const.tile([P, 2048], BF16, tag="wo", name="WO")   # (c, e)
    CB = const.tile([P, 2176], BF16, tag="cb", name="CB")
    FB = const.tile([P, 8], F32, tag="fb", name="FB")

    # sync queue carries the compute-gating chain; scalar queue the rest
    nc.sync.dma_start(WQ[:, 0:1024], d["wq"][:, 0:1024])
    nc.sync.dma_start(XT[:, 0:1024], d["xt"][:, 0:1024])
    nc.sync.dma_start(XT[:, 1024:2048], d["xt"][:, 1024:2048])
    nc.sync.dma_start(WK[:, 0:1024], d["wk"][:, 0:1024])
    nc.sync.dma_start(WV[:], d["wv"][:])
    nc.sync.dma_start(XT[:, 2048:4096], d["xt"][:, 2048:4096])
    nc.sync.dma_start(WO[:], d["wo"][:])
    nc.scalar.dma_start(FB[:], d["fb"][:])
    nc.scalar.dma_start(WQ[:, 1024:2048], d["wq"][:, 1024:2048])
    nc.scalar.dma_start(WK[:, 1024:2048], d["wk"][:, 1024:2048])
    nc.scalar.dma_start(CB[:], d["cb"][:])

    bq_t = FB[:, 0:4]
    bk_t = FB[:, 4:8]
    negi = CB[:, 0:128]
    tri2 = CB[:, 128:384]
    bvb = CB[:, 384:896]
    bob = CB[:, 896:1408]
    # Wo rows for the tail pair's upper head, re-homed at partition base 0
    wo3b = CB[0:DV, 1408:1920]
    # keep-mask (k <= q), duplicated for both heads of a pair
    keep2 = CB[:, 1920:2176]

    qt, kt, ot_sb = {}, {}, {}
    ot_tmp = None
    v_sb = [None] * 8

    def emit_qk_one(p, qc, which):
        W, bias, store, nm = ((WQ, bq_t, qt, "q") if which == "q"
                              else (WK, bk_t, kt, "k"))
        pp = ps.tile([P, 512], F32, tag="ps", name=f"{nm}p{p}{qc}")
        for c in range(EC):
            nc.tensor.matmul(
                pp[:], W[:, c * 512 + p * P:c * 512 + (p + 1) * P],
                XT[:, qc * 2048 + c * 512:qc * 2048 + (c + 1) * 512],
                start=(c == 0), stop=(c == EC - 1))
        t = const.tile([P, 512], BF16, tag=f"{nm}t{p}{qc}", name=f"{nm}t{p}{qc}")
        nc.scalar.activation(
            t[:], pp[:], mybir.ActivationFunctionType.Identity,
            bias=bias[:, p:p + 1])
        store[p, qc] = t

    def emit_qkt(p, qc):
        emit_qk_one(p, qc, "q")
        emit_qk_one(p, qc, "k")

    def emit_v(si):
        qc, sl = divmod(si, 4)
        vp = ps.tile([P, HD], F32, tag="ps", name=f"vp{si}")
        for c in range(EC):
            base = qc * 2048 + c * 512
            nc.tensor.matmul(
                vp[:], XT[:, base + sl * P:base + (sl + 1) * P],
                WV[:, c * 512:(c + 1) * 512],
                start=(c == 0), stop=(c == EC - 1))
        t = const.tile([P, H * 65], BF16, tag=f"v{si}", name=f"v{si}")
        t3 = t.rearrange("p (h c) -> p h c", c=65)
        nc.gpsimd.memset(t[:], 1.0)  # contiguous; leaves the per-head ones column
        nc.vector.tensor_add(
            t3[:, :, 0:DV],
            vp.rearrange("p (h c) -> p h c", c=DV),
            bvb.rearrange("p (h c) -> p h c", c=DV))
        v_sb[si] = t

    def emit_y(si, yp):
        yo = work.tile([P, E], BF16, tag="yo", name=f"yo{si}", bufs=4)
        nc.vector.scalar_tensor_tensor(yo[:], yp[:], 1.0, bob[:],
                                       mybir.AluOpType.mult, mybir.AluOpType.add)
        q = nc.sync if si % 2 == 0 else nc.scalar
        q.dma_start(d["y"][si * P:(si + 1) * P, :], yo[:])

    def emit_proj(si):
        qc, sl = divmod(si, 4)
        yp = ps.tile([P, E], F32, tag="ps", name=f"yp{si}")
        for p in range(NPAIR):
            nc.tensor.matmul(
                yp[:], ot_sb[p, qc][:, sl * P:(sl + 1) * P],
                WO[:, p * 512:(p + 1) * 512],
                start=(p == 0), stop=(p == NPAIR - 1), skip_group_check=True)
        emit_y(si, yp)

    def emit_proj_tail():
        # si 4..7: pairs 0/1/3 emitted first (they overlap the last-processed
        # pair-2 finalize chain), then pair 2 via two K=64 matmuls straight
        # from the scaled halves (no SBUF->SBUF DMA on the tail path).
        yps = {}
        for si in range(4, 8):
            pool, tag = (pb, "st") if si < 6 else (ps, "ps")
            yps[si] = pool.tile([P, E], F32, tag=tag, name=f"yp{si}")

        def part(si, p, start):  # one non-tail pair contribution, K=128
            sl = si - 4
            nc.tensor.matmul(
                yps[si][:], ot_sb[p, 1][:, sl * P:(sl + 1) * P],
                WO[:, p * 512:(p + 1) * 512],
                start=start, stop=False, skip_group_check=True)

        def split(si, hb):  # tail-pair halves, K=64 from the scaled temps
            sl = si - 4
            tp = TAIL_PAIR
            if hb == 0:
                nc.tensor.matmul(
                    yps[si][:], ot_sb[tp, 1][0:DV, sl * P:(sl + 1) * P],
                    WO[0:DV, tp * 512:(tp + 1) * 512],
                    start=False, stop=False, tile_position=(0, 0),
                    skip_group_check=True)
            else:
                nc.tensor.matmul(
                    yps[si][:], ot_tmp[:, sl * P:(sl + 1) * P],
                    wo3b[:],
                    start=False, stop=True, tile_position=(0, 0),
                    skip_group_check=True)

        others = [p for p in range(NPAIR) if p != TAIL_PAIR]
        for si in (4, 5, 6, 7):
            for i, p in enumerate(others):
                part(si, p, i == 0)
        for si_pair in ((4, 5), (6, 7)):
            for si in si_pair:
                split(si, 0)
            for si in si_pair:
                split(si, 1)
        for si in range(4, 8):
            emit_y(si, yps[si])

    otps = {}

    def finalize(p, qc, act_copy=False):
        # ---- O^T *= 1/denom (reciprocal needs an SBUF input on HW, so the
        # PSUM denom row is copied first -- on ACT for the last pair of a
        # phase, where ACT is about to idle and the chain gates psum reuse)
        otp = otps[p, qc]
        tail_pair = (qc == 1 and p == TAIL_PAIR)
        ot = const.tile([P, 512], BF16, tag=f"ot{p}{qc}", name=f"ot{p}{qc}")
        tmp = work.tile([DV, 512], BF16, tag="ottmp", name=f"ottmp{p}{qc}",
                        bufs=2)
        dst = {0: ot[0:DV, :], 1: tmp[:]}
        if act_copy:
            # last processed pair: this chain gates psum slot reuse (and the
            # tail splits), so run every stage in half-width pieces that
            # pipeline across ACT -> DVE -> GpSimd -> DVE
            rrow, rec, rb = {}, {}, {}
            for hb in (0, 1):
                rrow[hb] = work.tile([1, 512], F32, tag="rrow",
                                     name=f"rrow{p}{qc}{hb}", bufs=2)
                rec[hb] = work.tile([1, 512], F32, tag="rec",
                                    name=f"rec{p}{qc}{hb}", bufs=2)
                rb[hb] = work.tile([DV, 512], F32, tag="rb",
                                   name=f"rb{p}{qc}{hb}", bufs=2)
            halves = [(hb, slice(h * 256, (h + 1) * 256))
                      for hb in (0, 1) for h in (0, 1)]
            for hb, sl in halves:
                nc.scalar.activation(rrow[hb][:, sl], otp[hb][DV:DV + 1, sl],
                                     mybir.ActivationFunctionType.Copy)
            for hb, sl in halves:
                nc.vector.reciprocal_approx_fast(rec[hb][:, sl],
                                                 rrow[hb][:, sl])
            for hb, sl in halves:
                nc.gpsimd.partition_broadcast(rb[hb][:, sl], rec[hb][:, sl])
            for hb, sl in halves:
                nc.vector.tensor_mul(dst[hb][:, sl], otp[hb][0:DV, sl],
                                     rb[hb][:, sl])
        else:
            rb = {}
            for hb in (0, 1):
                rrow = work.tile([1, 512], F32, tag="rrow",
                                 name=f"rrow{p}{qc}{hb}", bufs=2)
                nc.vector.tensor_copy(rrow[:], otp[hb][DV:DV + 1, :])
                rec = work.tile([1, 512], F32, tag="rec",
                                name=f"rec{p}{qc}{hb}", bufs=2)
                nc.vector.reciprocal_approx_fast(rec[:], rrow[:])
                rb[hb] = work.tile([DV, 512], F32, tag="rb",
                                   name=f"rb{p}{qc}{hb}", bufs=2)
                nc.gpsimd.partition_broadcast(rb[hb][:], rec[:])
            nc.vector.tensor_mul(ot[0:DV, :], otp[0][0:DV, :], rb[0][:])
            nc.vector.tensor_mul(tmp[:], otp[1][0:DV, :], rb[1][:])
        # DVE cannot shift partitions: the upper head is scaled into a temp at
        # base 0, then SBUF->SBUF DMA'd into partitions 64-127 of the pair
        # tile (skipped for the tail pair: its projection reads the temp)
        if tail_pair:
            nonlocal ot_tmp
            ot_tmp = tmp
        else:
            nc.sync.dma_start(ot[DV:P, :], tmp[:])
        ot_sb[p, qc] = ot

    def attn_phase(qc):
        # flat block stream across all pairs with a TWO-block AV lookahead:
        # AV(i) issues after scores(i+1), scores(i+2) are queued, so the PE
        # has ~2 score blocks of runway over the exp latency
        n_ki = 4 * (qc + 1)
        order = P1_ORDER if qc == 1 else tuple(range(NPAIR))
        blocks = [(p, ki) for p in order for ki in range(n_ki)]
        stps, stes = {}, {}

        def emit_score(p, ki):
            kc, kl = divmod(ki, 4)
            diag = (ki * P - qc * 512) >= 0
            off = max(ki * P - qc * 512, 0)
            # qc=1 diag blocks are masked post-exp on DVE instead; qc=0 keeps
            # the PE fixup since the DVE would gate the exp pipeline
            pe_fix = diag and qc == 0
            stp = pb.tile([P, 1024], F32, tag="st", name=f"st{p}{qc}{ki}")
            for hb in (0, 1):
                hp = slice(hb * DK, (hb + 1) * DK)
                nc.tensor.matmul(
                    stp[:, hb * 512 + off:(hb + 1) * 512],
                    kt[p, kc][hp, kl * P:(kl + 1) * P],
                    qt[p, qc][hp, off:],
                    start=True, stop=not pe_fix, tile_position=(hb * DK, 0),
                    skip_group_check=True)
            if pe_fix:
                for hb in (0, 1):
                    nc.tensor.matmul(
                        stp[:, hb * 512 + off:hb * 512 + off + P],
                        negi[:], tri2[:, 0:P],
                        start=False, stop=True, skip_group_check=True)
            stps[p, ki] = (stp, off, diag and qc == 1)

        def emit_exp(p, ki):
            stp, off, dve_mask = stps[p, ki]
            ste = work.tile([P, 1024], BF16, tag="ste", name=f"ste{p}{qc}{ki}",
                            bufs=3)
            if off == 0:
                nc.scalar.activation(
                    ste[:], stp[:], mybir.ActivationFunctionType.Exp,
                    scale=0.125)
            else:
                stp3 = stp.rearrange("p (h q) -> p h q", h=2)[:, :, off:]
                ste3 = ste.rearrange("p (h q) -> p h q", h=2)[:, :, off:]
                nc.scalar.activation(
                    ste3, stp3, mybir.ActivationFunctionType.Exp, scale=0.125)
            if dve_mask:
                # zero exp'd scores above the diagonal (both heads at once)
                sv = ste.rearrange("p (h q) -> p h q", h=2)[:, :, off:off + P]
                nc.vector.tensor_mul(
                    sv, sv, keep2.rearrange("p (h q) -> p h q", h=2))
            stes[p, ki] = (ste, off)

        def emit_av(p, ki):
            ste, off = stes[p, ki]
            st_f, sp_f = (ki == 0), (ki == n_ki - 1)
            for hb in (0, 1):
                h = 2 * p + hb
                nc.tensor.matmul(
                    otps[p, qc][hb][:, off:], v_sb[ki][:, h * 65:h * 65 + 65],
                    ste[:, hb * 512 + off:(hb + 1) * 512],
                    start=st_f, stop=sp_f, skip_group_check=True)

        for idx, (p, ki) in enumerate(blocks):
            if ki == 0:
                otps[p, qc] = [ps.tile([DV + 1, 512], F32, tag="ps",
                                       name=f"otp{p}{qc}{hb}") for hb in (0, 1)]
            emit_score(p, ki)
            emit_exp(p, ki)
            if idx >= 1:
                pp, pk = blocks[idx - 1]
                emit_av(pp, pk)
                if pk == n_ki - 1:
                    finalize(pp, qc, act_copy=(pp == order[-1]))
        emit_av(*blocks[-1])
        finalize(order[-1], qc, act_copy=True)

    # ---- emission schedule (ps-tag rotation keeps otp pairs ping-ponging
    # between bank pairs) ----
    for p in range(NPAIR):            # ps allocs 0-7; all QTs first so the
        emit_qk_one(p, 0, "q")        # PE never queues behind the WK DMA
    for p in range(NPAIR):
        emit_qk_one(p, 0, "k")
    for si in range(4):               # 8-11
        emit_v(si)
    attn_phase(0)                     # 12-19 -> slots (0,1)/(2,3)/(0,1)/(2,3)
    emit_qkt(0, 1)                    # 20-21
    emit_qkt(1, 1)                    # 22-23
    emit_v(4)                         # 24
    emit_v(5)                         # 25
    emit_qkt(2, 1)                    # 26-27
    emit_qkt(3, 1)                    # 28-29
    emit_v(6)                         # 30
    emit_v(7)                         # 31
    for si in range(4):               # (needs all qc=0 ot, finalized)
        emit_proj(si)
    attn_phase(1)                     # 36-43 -> slots (0,1)/(2,3)/(0,1)/(2,3)
    emit_proj_tail()                  # yp6/7 at ps 44,45 -> slots 0,1


def _build():
    nc = bacc.Bacc("TRN2", target_bir_lowering=False, debug=False)
    d = {
        "xt": nc.dram_tensor("xt", [P, 4096], BF16, kind="ExternalInput").ap(),
        "wq": nc.dram_tensor("wq", [P, 2048], BF16, kind="ExternalInput").ap(),
        "wk": nc.dram_tensor("wk", [P, 2048], BF16, kind="ExternalInput").ap(),
        "wv": nc.dram_tensor("wv", [P, 2048], BF16, kind="ExternalInput").ap(),
        "wo": nc.dram_tensor("wo", [P, 2048], BF16, kind="ExternalInput").ap(),
        "cb": nc.dram_tensor("cb", [P, 2176], BF16, kind="ExternalInput").ap(),
        "fb": nc.dram_tensor("fb", [P, 8], F32, kind="ExternalInput").ap(),
        "y": nc.dram_tensor("y", [S, E], BF16, kind="ExternalOutput").ap(),
    }
    with tile.TileContext(nc) as tc:
        with tc.tile_pool(name="const", bufs=1) as const, \
             tc.tile_pool(name="work", bufs=3) as work, \
             tc.tile_pool(name="ps", bufs=4, space="PSUM") as ps, \
             tc.tile_pool(name="pb", bufs=2, space="PSUM") as pb:
            _body(nc, tc, const, work, ps, pb, d)
    nc.compile()
    return nc


def get_nc():
    global _COMPILED
    if _COMPILED is None:
        _COMPILED = _build()
    return _COMPILED


def _prep_in_maps(X, Wq, bq, Wk, bk, Wv, bv, Wo, bo):
    f = np.float32
    bf = ml_dtypes.bfloat16

    def wslab(W):  # [H,E,Dk] -> [128, c*512 + (h*64+d)]
        Wr = np.transpose(np.asarray(W, f), (1, 0, 2)).reshape(E, HD)
        return np.ascontiguousarray(
            Wr.reshape(EC, P, HD).transpose(1, 0, 2).reshape(P, EC * HD).astype(bf))

    shared = {
        "wq": wslab(Wq),
        "wk": wslab(Wk),
        "wv": wslab(Wv),
        "wo": np.ascontiguousarray(
            np.asarray(Wo, f).reshape(EC, P, E).transpose(1, 0, 2)
            .reshape(P, EC * E).astype(bf)),
    }
    bq_t = np.asarray(bq, f).reshape(HD).reshape(NPAIR, P).T
    bk_t = np.asarray(bk, f).reshape(HD).reshape(NPAIR, P).T
    bvb = np.broadcast_to(np.asarray(bv, f).reshape(1, HD), (P, HD)).astype(bf)
    bob = np.broadcast_to(np.asarray(bo, f).reshape(1, E), (P, E)).astype(bf)
    kk = np.arange(P)[:, None]
    jj = np.arange(P)[None, :]
    shared["fb"] = np.ascontiguousarray(
        np.concatenate([bq_t, bk_t], axis=1).astype(f))
    negi = (np.eye(P, dtype=f) * NEG).astype(bf)
    tri01 = (kk > jj).astype(bf)
    # Wo rows for the tail pair's upper head, re-homed to partitions 0:64
    wo3b = np.zeros((P, E), dtype=bf)
    wo3b[0:DV] = np.asarray(Wo, f)[(2 * TAIL_PAIR + 1) * DV:
                                   (2 * TAIL_PAIR + 2) * DV, :].astype(bf)
    keep01 = (kk <= jj).astype(bf)
    shared["cb"] = np.ascontiguousarray(
        np.concatenate([negi, tri01, tri01, bvb, bob, wo3b, keep01, keep01],
                       axis=1))

    Xf = np.asarray(X, f)
    in_maps = []
    for b in range(B):
        m = dict(shared)
        # xt slab: [128, qc*2048 + c*512 + s']
        m["xt"] = np.ascontiguousarray(
            Xf[b].T.reshape(EC, P, 2, 512).transpose(1, 2, 0, 3)
            .reshape(P, 4096).astype(bf))
        in_maps.append(m)
    return in_maps


def kernel(X, Wq, bq, Wk, bk, Wv, bv, Wo, bo):
    nc = get_nc()
    in_maps = _prep_in_maps(X, Wq, bq, Wk, bk, Wv, bv, Wo, bo)
    res = bass_utils.run_bass_kernel_spmd(nc, in_maps, core_ids=list(range(NCORES)))
    return np.stack([res.results[b]["y"] for b in range(B)], axis=0).astype(np.float32)


def run_traced(X, Wq, bq, Wk, bk, Wv, bv, Wo, bo):
    """Like kernel() but with NTFF profiling; returns (out, exec_time_ns)."""
    nc = get_nc()
    in_maps = _prep_in_maps(X, Wq, bq, Wk, bk, Wv, bv, Wo, bo)
    res = bass_utils.run_bass_kernel_spmd(
        nc, in_maps, core_ids=list(range(NCORES)), trace=True)
    out = np.stack([res.results[b]["y"] for b in range(B)], axis=0).astype(np.float32)
    return out, res.exec_time_ns
